# revision 3
# baseline (speedup 1.0000x reference)
"""DeepSeekMoE (2 shared + 8 routed experts, top-2) on 8 Trainium2 NeuronCores.

Strategy (expert-parallel, per sharding hint):
  - Routed experts: core e owns expert e. Host computes router logits/top-2
    (0.01% of FLOPs), gathers each expert's tokens into a fixed-capacity
    buffer (SPMD: same shapes on every core), cores run the dense expert MLP,
    host applies gate weights and scatter-adds (the "all-to-all" combine).
  - Shared experts: data-parallel. Core c processes tokens [c*512,(c+1)*512)
    through both shared experts, streaming the (replicated) shared weights.
  - Router: each core also computes logits for its own 512-token slice on
    device; those logits are the returned logits.

All matmuls run in [feature, token] layout: stationary weights [K=128, M=128]
blocks, moving activations [K=128, N<=512], bf16 inputs with f32 PSUM
accumulation. Weights are pre-packed on host into contiguous-DMA block
layouts. Layer-1 bias is fused into the ReLU activation on device; layer-2
bias terms (zero in this model, but handled generally) are added on host.
"""

import numpy as np
import ml_dtypes

import concourse.bass as bass
import concourse.tile as tile
from concourse import bacc, mybir
from concourse.bass_utils import run_bass_kernel_spmd

BF16 = ml_dtypes.bfloat16

H = 2048          # hidden dim
F = 8192          # ffn dim
E = 8             # routed experts
S = 2             # shared experts
B, SEQ = 2, 2048
T = B * SEQ       # 4096 tokens
P = 128
DO = H // P       # 16 d-blocks
FO = F // P       # 64 f-blocks
TSH = T // 8      # 512 shared tokens per core
NT = 512          # token tile (matmul moving dim)

_PROG_CACHE = {}
LAST_RESULT = None  # test.py introspection


def _t_chunks(C):
    chunks = []
    t0 = 0
    while t0 < C:
        nt = min(NT, C - t0)
        chunks.append((t0, nt))
        t0 += nt
    return chunks


def _build_program(C):
    """One SPMD program, identical on all 8 cores."""
    nc = bacc.Bacc("TRN2", target_bir_lowering=False, debug=False)
    bf = mybir.dt.bfloat16
    f32 = mybir.dt.float32

    def inp(name, shape, dt=bf):
        return nc.dram_tensor(name, list(shape), dt, kind="ExternalInput").ap()

    def outp(name, shape, dt=f32):
        return nc.dram_tensor(name, list(shape), dt, kind="ExternalOutput").ap()

    xsT = inp("xsT", [P, DO, TSH])            # shared tokens, [di, do, t]
    xrT = inp("xrT", [P, DO, C])              # routed tokens for this expert
    w1s = inp("w1s", [S, FO, P, DO, P])       # (s, fo, di, do, fi)
    w2s = inp("w2s", [S, DO, P, FO, P])       # (s, do, fi, fo, di)
    w1e = inp("w1e", [FO, P, DO, P])
    w2e = inp("w2e", [DO, P, FO, P])
    b1s = inp("b1s", [P, S, FO], f32)         # (fi, s, fo)
    b1e = inp("b1e", [P, FO], f32)
    wgp = inp("wgp", [P, DO, E])              # (di, do, e)

    lg_o = outp("lg_o", [E, TSH])             # logits for this core's tokens
    ys_o = outp("ys_o", [S, DO, P, TSH])      # shared outputs, [d, t] layout
    yr_o = outp("yr_o", [DO, P, C])           # routed output

    with tile.TileContext(nc) as tc:
        with (
            tc.tile_pool(name="xpool", bufs=1) as xpool,
            tc.tile_pool(name="cpool", bufs=1) as cpool,
            tc.tile_pool(name="w1p", bufs=3) as w1p,
            tc.tile_pool(name="w2p", bufs=2) as w2p,
            tc.tile_pool(name="hp", bufs=FO) as hp,
            tc.tile_pool(name="yp", bufs=4) as yp,
            tc.tile_pool(name="psp", bufs=4, space="PSUM") as psp,
            tc.tile_pool(name="psg", bufs=1, space="PSUM") as psg,
        ):
            # resident activations / constants
            xs_sb = xpool.tile([P, DO, TSH], bf, tag="xs")
            nc.sync.dma_start(xs_sb[:], xsT[:])
            xr_sb = xpool.tile([P, DO, C], bf, tag="xr")
            nc.sync.dma_start(xr_sb[:], xrT[:])
            b1s_sb = cpool.tile([P, S, FO], f32, tag="b1s")
            nc.sync.dma_start(b1s_sb[:], b1s[:])
            b1e_sb = cpool.tile([P, FO], f32, tag="b1e")
            nc.sync.dma_start(b1e_sb[:], b1e[:])
            wg_sb = cpool.tile([P, DO, E], bf, tag="wg")
            nc.sync.dma_start(wg_sb[:], wgp[:])

            # ---- router logits for this core's 512 tokens ----
            ps_g = psg.tile([E, TSH], f32)
            for do in range(DO):
                nc.tensor.matmul(
                    ps_g[:],
                    wg_sb[:, do, :],
                    xs_sb[:, do, :],
                    start=(do == 0),
                    stop=(do == DO - 1),
                )
            lg_sb = yp.tile([E, TSH], f32, tag="lg")
            nc.vector.tensor_copy(lg_sb[:], ps_g[:])
            nc.sync.dma_start(lg_o[:], lg_sb[:])

            # ---- expert-MLP units ----
            # unit = (w1 dram [FO,P,DO,P], w2 dram [DO,P,FO,P], bias AP fn,
            #         x tile, t0, nt, out dram slice fn)
            units = []
            for s in range(S):
                units.append(
                    (
                        w1s[s],
                        w2s[s],
                        lambda fo, s=s: b1s_sb[:, s, fo : fo + 1],
                        xs_sb,
                        0,
                        TSH,
                        lambda do, s=s: ys_o[s, do],
                    )
                )
            for t0, nt in _t_chunks(C):
                units.append(
                    (
                        w1e,
                        w2e,
                        lambda fo: b1e_sb[:, fo : fo + 1],
                        xr_sb,
                        t0,
                        nt,
                        lambda do, t0=t0, nt=nt: yr_o[do, :, t0 : t0 + nt],
                    )
                )

            for w1d, w2d, bias_fn, x_sb, t0, nt, out_fn in units:
                h_tiles = []
                for fo in range(FO):
                    w1_t = w1p.tile([P, DO, P], bf, tag="w1")
                    nc.sync.dma_start(w1_t[:], w1d[fo])
                    ps = psp.tile([P, NT], f32, tag="ps")
                    for do in range(DO):
                        nc.tensor.matmul(
                            ps[:, :nt],
                            w1_t[:, do, :],
                            x_sb[:, do, t0 : t0 + nt],
                            start=(do == 0),
                            stop=(do == DO - 1),
                        )
                    h_t = hp.tile([P, NT], bf, tag="h")
                    nc.scalar.activation(
                        h_t[:, :nt],
                        ps[:, :nt],
                        mybir.ActivationFunctionType.Relu,
                        bias=bias_fn(fo),
                    )
                    h_tiles.append(h_t)
                for do in range(DO):
                    w2_t = w2p.tile([P, FO, P], bf, tag="w2")
                    nc.sync.dma_start(w2_t[:], w2d[do])
                    ps2 = psp.tile([P, NT], f32, tag="ps")
                    for fo in range(FO):
                        nc.tensor.matmul(
                            ps2[:, :nt],
                            w2_t[:, fo, :],
                            h_tiles[fo][:, :nt],
                            start=(fo == 0),
                            stop=(fo == FO - 1),
                        )
                    y_t = yp.tile([P, NT], f32, tag="y")
                    nc.vector.tensor_copy(y_t[:, :nt], ps2[:, :nt])
                    nc.sync.dma_start(out_fn(do), y_t[:, :nt])

    nc.compile()
    return nc


def _get_program(C):
    if C not in _PROG_CACHE:
        _PROG_CACHE[C] = _build_program(C)
    return _PROG_CACHE[C]


def _pack_w1(w):
    # [H, F] -> (fo, di, do, fi); SBUF tile per fo: [di, do*fi] contiguous
    return np.ascontiguousarray(
        w.reshape(DO, P, FO, P).transpose(2, 1, 0, 3)
    )


def _pack_w2(w):
    # [F, H] -> (do, fi, fo, di)
    return np.ascontiguousarray(
        w.reshape(FO, P, DO, P).transpose(2, 1, 0, 3)
    )


def _pack_xT(xt_bf):
    # [n, H] bf16 -> [di, do, n]
    n = xt_bf.shape[0]
    return np.ascontiguousarray(
        xt_bf.T.reshape(DO, P, n).transpose(1, 0, 2)
    )


def kernel(
    x,
    ws_w1,
    ws_b1,
    ws_w2,
    ws_b2,
    we_w1,
    we_b1,
    we_w2,
    we_b2,
    wg,
    top_k,
    trace=False,
    run_kwargs=None,
    **_unused,
):
    global LAST_RESULT
    x = np.asarray(x, dtype=np.float32)
    ws_w1 = np.asarray(ws_w1, dtype=np.float32)
    ws_b1 = np.asarray(ws_b1, dtype=np.float32)
    ws_w2 = np.asarray(ws_w2, dtype=np.float32)
    ws_b2 = np.asarray(ws_b2, dtype=np.float32)
    we_w1 = np.asarray(we_w1, dtype=np.float32)
    we_b1 = np.asarray(we_b1, dtype=np.float32)
    we_w2 = np.asarray(we_w2, dtype=np.float32)
    we_b2 = np.asarray(we_b2, dtype=np.float32)
    wg = np.asarray(wg, dtype=np.float32)
    k = int(top_k)

    Bx, Sx, Hx = x.shape
    Tx = Bx * Sx
    assert (Tx, Hx) == (T, H), f"unexpected shape {x.shape}"
    xt = x.reshape(Tx, Hx)

    # ---- host routing decision (dispatch plan only; logits returned from HW)
    logits_h = xt @ wg
    m = logits_h.max(axis=1, keepdims=True)
    p = np.exp(logits_h - m)
    probs = p / p.sum(axis=1, keepdims=True)
    order = np.argsort(-probs, axis=1, kind="stable")[:, :k]
    gates = np.zeros_like(probs)
    np.put_along_axis(gates, order, np.take_along_axis(probs, order, 1), 1)
    sel = np.zeros((Tx, E), dtype=bool)
    np.put_along_axis(sel, order, True, 1)
    idx_e = [np.nonzero(sel[:, e])[0] for e in range(E)]
    n_e = [len(ix) for ix in idx_e]
    C = max(P, int(-(-max(n_e) // P)) * P)

    # ---- pack per-core inputs
    xt_bf = xt.astype(BF16)
    w1s_h = np.stack([_pack_w1(ws_w1[s].astype(BF16)) for s in range(S)])
    w2s_h = np.stack([_pack_w2(ws_w2[s].astype(BF16)) for s in range(S)])
    b1s_h = np.ascontiguousarray(
        ws_b1.reshape(S, FO, P).transpose(2, 0, 1)
    ).astype(np.float32)
    wg_h = np.ascontiguousarray(
        wg.astype(BF16).reshape(DO, P, E).transpose(1, 0, 2)
    )

    in_maps = []
    for c in range(E):
        xr = np.zeros((C, H), dtype=BF16)
        xr[: n_e[c]] = xt_bf[idx_e[c]]
        in_maps.append(
            {
                "xsT": _pack_xT(xt_bf[c * TSH : (c + 1) * TSH]),
                "xrT": _pack_xT(xr),
                "w1s": w1s_h,
                "w2s": w2s_h,
                "w1e": _pack_w1(we_w1[c].astype(BF16)),
                "w2e": _pack_w2(we_w2[c].astype(BF16)),
                "b1s": b1s_h,
                "b1e": np.ascontiguousarray(
                    we_b1[c].reshape(FO, P).T
                ).astype(np.float32),
                "wgp": wg_h,
            }
        )

    nc = _get_program(C)
    res = run_bass_kernel_spmd(
        nc, in_maps, list(range(8)), trace=trace, **(run_kwargs or {})
    )
    LAST_RESULT = res

    # ---- combine on host
    out = np.zeros((Tx, H), dtype=np.float32)
    logits = np.empty((Tx, E), dtype=np.float32)
    for c in range(E):
        r = res.results[c]
        logits[c * TSH : (c + 1) * TSH] = r["lg_o"].T
        ys = r["ys_o"].sum(axis=0)  # [DO, P, TSH]
        out[c * TSH : (c + 1) * TSH] += (
            ys.reshape(H, TSH).T
        )
        if n_e[c]:
            yr = r["yr_o"].reshape(H, C).T[: n_e[c]]  # [n_e, H]
            out[idx_e[c]] += gates[idx_e[c], c : c + 1] * yr
    out += ws_b2.sum(axis=0)[None, :]
    out += gates @ we_b2
    return out.reshape(Bx, Sx, Hx), logits


# revision 9
# speedup vs baseline: 1.1085x; 1.1085x over previous
"""DeepSeekMoE (2 shared + 8 routed experts, top-2) on 8 Trainium2 NeuronCores.

Strategy (expert-parallel, per sharding hint):
  - Routed experts: core e owns expert e. Host computes router logits/top-2
    (0.01% of FLOPs), gathers each expert's tokens into a fixed-capacity
    buffer (SPMD: same shapes on every core), cores run the dense expert MLP,
    host applies gate weights and scatter-adds (the "all-to-all" combine).
  - Shared experts: data-parallel. Core c processes tokens [c*512,(c+1)*512)
    through both shared experts, streaming the (replicated) shared weights.
  - Router: each core also computes logits for its own 512-token slice on
    device; those logits are the returned logits.

All matmuls run in [feature, token] layout: stationary weights [K=128, M=128]
blocks, moving activations [K=128, N<=512], bf16 inputs with f32 PSUM
accumulation. Weights are pre-packed on host into contiguous-DMA block
layouts. Layer-1 bias is fused into the ReLU activation on device; layer-2
bias terms (zero in this model, but handled generally) are added on host.
"""

import numpy as np
import ml_dtypes

import concourse.bass as bass
import concourse.tile as tile
from concourse import bacc, mybir
from concourse.bass_utils import run_bass_kernel_spmd

BF16 = ml_dtypes.bfloat16

H = 2048          # hidden dim
F = 8192          # ffn dim
E = 8             # routed experts
S = 2             # shared experts
B, SEQ = 2, 2048
T = B * SEQ       # 4096 tokens
P = 128
DO = H // P       # 16 d-blocks
FO = F // P       # 64 f-blocks
TSH = T // 8      # 512 shared tokens per core
NT = 512          # token tile (matmul moving dim)

_PROG_CACHE = {}
LAST_RESULT = None  # test.py introspection


def _t_chunks(C):
    # equal-width chunks <= NT: a narrow tail chunk pays the full
    # per-matmul issue floor for few columns and goes DMA-bound.
    n = -(-C // NT)
    base, rem = divmod(C, n)
    chunks = []
    t0 = 0
    for i in range(n):
        nt = base + (1 if i < rem else 0)
        chunks.append((t0, nt))
        t0 += nt
    return chunks


def _build_program(C):
    """One SPMD program, identical on all 8 cores."""
    nc = bacc.Bacc("TRN2", target_bir_lowering=False, debug=False)
    bf = mybir.dt.bfloat16
    f32 = mybir.dt.float32

    def inp(name, shape, dt=bf):
        return nc.dram_tensor(name, list(shape), dt, kind="ExternalInput").ap()

    def outp(name, shape, dt=f32):
        return nc.dram_tensor(name, list(shape), dt, kind="ExternalOutput").ap()

    xsT = inp("xsT", [P, DO, TSH])            # shared tokens, [di, do, t]
    xrT = inp("xrT", [P, DO, C])              # routed tokens for this expert
    w1s = inp("w1s", [S, FO, P, DO, P])       # (s, fo, di, do, fi)
    w2s = inp("w2s", [S, DO, P, FO, P])       # (s, do, fi, fo, di)
    w1e = inp("w1e", [FO, P, DO, P])
    w2e = inp("w2e", [DO, P, FO, P])
    b1s = inp("b1s", [P, S, FO], f32)         # (fi, s, fo)
    b1e = inp("b1e", [P, FO], f32)
    wgp = inp("wgp", [P, DO, E])              # (di, do, e)

    lg_o = outp("lg_o", [E, TSH])             # logits for this core's tokens
    ys_o = outp("ys_o", [S, DO, P, TSH])      # shared outputs, [d, t] layout
    yr_o = outp("yr_o", [DO, P, C])           # routed output

    with tile.TileContext(nc) as tc:
        with (
            tc.tile_pool(name="xpool", bufs=1) as xpool,
            tc.tile_pool(name="cpool", bufs=1) as cpool,
            tc.tile_pool(name="w1p", bufs=4) as w1p,
            tc.tile_pool(name="w2p", bufs=3) as w2p,
            tc.tile_pool(name="hp", bufs=FO) as hp,
            tc.tile_pool(name="yp", bufs=4) as yp,
            tc.tile_pool(name="psp", bufs=4, space="PSUM") as psp,
            tc.tile_pool(name="psg", bufs=1, space="PSUM") as psg,
        ):
            # resident activations / constants
            xs_sb = xpool.tile([P, DO, TSH], bf, tag="xs")
            nc.sync.dma_start(xs_sb[:], xsT[:])
            xr_sb = xpool.tile([P, DO, C], bf, tag="xr")
            b1s_sb = cpool.tile([P, S, FO], f32, tag="b1s")
            nc.sync.dma_start(b1s_sb[:], b1s[:])
            b1e_sb = cpool.tile([P, FO], f32, tag="b1e")
            nc.sync.dma_start(b1e_sb[:], b1e[:])
            wg_sb = cpool.tile([P, DO, E], bf, tag="wg")
            nc.sync.dma_start(wg_sb[:], wgp[:])

            # ---- router logits for this core's 512 tokens ----
            ps_g = psg.tile([E, TSH], f32)
            for do in range(DO):
                nc.tensor.matmul(
                    ps_g[:],
                    wg_sb[:, do, :],
                    xs_sb[:, do, :],
                    start=(do == 0),
                    stop=(do == DO - 1),
                )
            lg_sb = yp.tile([E, TSH], f32, tag="lg")
            nc.vector.tensor_copy(lg_sb[:], ps_g[:])
            nc.sync.dma_start(lg_o[:], lg_sb[:])

            # ---- expert-MLP units ----
            # unit = (w1 dram [FO,P,DO,P], w2 dram [DO,P,FO,P], bias AP fn,
            #         x tile, t0, nt, out dram slice fn)
            shared_units = [
                (
                    w1s[s],
                    w2s[s],
                    lambda fo, s=s: b1s_sb[:, s, fo : fo + 1],
                    xs_sb,
                    0,
                    TSH,
                    lambda do, s=s: ys_o[s, do],
                )
                for s in range(S)
            ]
            routed_units = [
                (
                    w1e,
                    w2e,
                    lambda fo: b1e_sb[:, fo : fo + 1],
                    xr_sb,
                    t0,
                    nt,
                    lambda do, t0=t0, nt=nt: yr_o[do, :, t0 : t0 + nt],
                )
                for t0, nt in _t_chunks(C)
            ]

            def emit_unit(w1d, w2d, bias_fn, x_sb, t0, nt, out_fn):
                h_tiles = []
                for fo in range(FO):
                    w1_t = w1p.tile([P, DO, P], bf, tag="w1")
                    nc.sync.dma_start(w1_t[:], w1d[fo])
                    ps = psp.tile([P, NT], f32, tag="ps")
                    for do in range(DO):
                        nc.tensor.matmul(
                            ps[:, :nt],
                            w1_t[:, do, :],
                            x_sb[:, do, t0 : t0 + nt],
                            start=(do == 0),
                            stop=(do == DO - 1),
                        )
                    h_t = hp.tile([P, NT], bf, tag="h")
                    nc.scalar.activation(
                        h_t[:, :nt],
                        ps[:, :nt],
                        mybir.ActivationFunctionType.Relu,
                        bias=bias_fn(fo),
                    )
                    h_tiles.append(h_t)
                for do in range(DO):
                    w2_t = w2p.tile([P, FO, P], bf, tag="w2")
                    nc.sync.dma_start(w2_t[:], w2d[do])
                    ps2 = psp.tile([P, NT], f32, tag="ps")
                    for fo in range(FO):
                        nc.tensor.matmul(
                            ps2[:, :nt],
                            w2_t[:, fo, :],
                            h_tiles[fo][:, :nt],
                            start=(fo == 0),
                            stop=(fo == FO - 1),
                        )
                    y_t = yp.tile([P, NT], f32, tag="y")
                    nc.vector.tensor_copy(y_t[:, :nt], ps2[:, :nt])
                    nc.sync.dma_start(out_fn(do), y_t[:, :nt])

            for u in shared_units:
                emit_unit(*u)
            # xr load deferred here so it doesn't queue ahead of the
            # shared units' weight streams (kills the startup PE stall)
            nc.sync.dma_start(xr_sb[:], xrT[:])
            for u in routed_units:
                emit_unit(*u)

    nc.compile()
    return nc


def _get_program(C):
    if C not in _PROG_CACHE:
        _PROG_CACHE[C] = _build_program(C)
    return _PROG_CACHE[C]


def _pack_w1(w):
    # [H, F] -> (fo, di, do, fi); SBUF tile per fo: [di, do*fi] contiguous
    return np.ascontiguousarray(
        w.reshape(DO, P, FO, P).transpose(2, 1, 0, 3)
    )


def _pack_w2(w):
    # [F, H] -> (do, fi, fo, di)
    return np.ascontiguousarray(
        w.reshape(FO, P, DO, P).transpose(2, 1, 0, 3)
    )


def _pack_xT(xt_bf):
    # [n, H] bf16 -> [di, do, n]
    n = xt_bf.shape[0]
    return np.ascontiguousarray(
        xt_bf.T.reshape(DO, P, n).transpose(1, 0, 2)
    )


def kernel(
    x,
    ws_w1,
    ws_b1,
    ws_w2,
    ws_b2,
    we_w1,
    we_b1,
    we_w2,
    we_b2,
    wg,
    top_k,
    trace=False,
    run_kwargs=None,
    **_unused,
):
    global LAST_RESULT
    x = np.asarray(x, dtype=np.float32)
    ws_w1 = np.asarray(ws_w1, dtype=np.float32)
    ws_b1 = np.asarray(ws_b1, dtype=np.float32)
    ws_w2 = np.asarray(ws_w2, dtype=np.float32)
    ws_b2 = np.asarray(ws_b2, dtype=np.float32)
    we_w1 = np.asarray(we_w1, dtype=np.float32)
    we_b1 = np.asarray(we_b1, dtype=np.float32)
    we_w2 = np.asarray(we_w2, dtype=np.float32)
    we_b2 = np.asarray(we_b2, dtype=np.float32)
    wg = np.asarray(wg, dtype=np.float32)
    k = int(top_k)

    Bx, Sx, Hx = x.shape
    Tx = Bx * Sx
    assert (Tx, Hx) == (T, H), f"unexpected shape {x.shape}"
    xt = x.reshape(Tx, Hx)

    # ---- host routing decision (dispatch plan only; logits returned from HW)
    logits_h = xt @ wg
    m = logits_h.max(axis=1, keepdims=True)
    p = np.exp(logits_h - m)
    probs = p / p.sum(axis=1, keepdims=True)
    order = np.argsort(-probs, axis=1, kind="stable")[:, :k]
    gates = np.zeros_like(probs)
    np.put_along_axis(gates, order, np.take_along_axis(probs, order, 1), 1)
    sel = np.zeros((Tx, E), dtype=bool)
    np.put_along_axis(sel, order, True, 1)
    idx_e = [np.nonzero(sel[:, e])[0] for e in range(E)]
    n_e = [len(ix) for ix in idx_e]
    # capacity: equal 4-aligned chunks of <= NT covering the largest expert
    mx = max(max(n_e), 16)
    nch = -(-mx // NT)
    C = -(-mx // (4 * nch)) * 4 * nch

    # ---- pack per-core inputs
    xt_bf = xt.astype(BF16)
    w1s_h = np.stack([_pack_w1(ws_w1[s].astype(BF16)) for s in range(S)])
    w2s_h = np.stack([_pack_w2(ws_w2[s].astype(BF16)) for s in range(S)])
    b1s_h = np.ascontiguousarray(
        ws_b1.reshape(S, FO, P).transpose(2, 0, 1)
    ).astype(np.float32)
    wg_h = np.ascontiguousarray(
        wg.astype(BF16).reshape(DO, P, E).transpose(1, 0, 2)
    )

    in_maps = []
    for c in range(E):
        xr = np.zeros((C, H), dtype=BF16)
        xr[: n_e[c]] = xt_bf[idx_e[c]]
        in_maps.append(
            {
                "xsT": _pack_xT(xt_bf[c * TSH : (c + 1) * TSH]),
                "xrT": _pack_xT(xr),
                "w1s": w1s_h,
                "w2s": w2s_h,
                "w1e": _pack_w1(we_w1[c].astype(BF16)),
                "w2e": _pack_w2(we_w2[c].astype(BF16)),
                "b1s": b1s_h,
                "b1e": np.ascontiguousarray(
                    we_b1[c].reshape(FO, P).T
                ).astype(np.float32),
                "wgp": wg_h,
            }
        )

    nc = _get_program(C)
    res = run_bass_kernel_spmd(
        nc, in_maps, list(range(8)), trace=trace, **(run_kwargs or {})
    )
    LAST_RESULT = res

    # ---- combine on host
    out = np.zeros((Tx, H), dtype=np.float32)
    logits = np.empty((Tx, E), dtype=np.float32)
    for c in range(E):
        r = res.results[c]
        logits[c * TSH : (c + 1) * TSH] = r["lg_o"].T
        ys = r["ys_o"].sum(axis=0)  # [DO, P, TSH]
        out[c * TSH : (c + 1) * TSH] += (
            ys.reshape(H, TSH).T
        )
        if n_e[c]:
            yr = r["yr_o"].reshape(H, C).T[: n_e[c]]  # [n_e, H]
            out[idx_e[c]] += gates[idx_e[c], c : c + 1] * yr
    out += ws_b2.sum(axis=0)[None, :]
    out += gates @ we_b2
    return out.reshape(Bx, Sx, Hx), logits


# revision 11
# speedup vs baseline: 1.1093x; 1.0007x over previous
"""DeepSeekMoE (2 shared + 8 routed experts, top-2) on 8 Trainium2 NeuronCores.

Strategy (expert-parallel, per sharding hint):
  - Routed experts: core e owns expert e. Host computes router logits/top-2
    (0.01% of FLOPs), gathers each expert's tokens into a fixed-capacity
    buffer (SPMD: same shapes on every core), cores run the dense expert MLP,
    host applies gate weights and scatter-adds (the "all-to-all" combine).
  - Shared experts: data-parallel. Core c processes tokens [c*512,(c+1)*512)
    through both shared experts, streaming the (replicated) shared weights.
  - Router: each core also computes logits for its own 512-token slice on
    device; those logits are the returned logits.

All matmuls run in [feature, token] layout: stationary weights [K=128, M=128]
blocks, moving activations [K=128, N<=512], bf16 inputs with f32 PSUM
accumulation. Weights are pre-packed on host into contiguous-DMA block
layouts. Layer-1 bias is fused into the ReLU activation on device; layer-2
bias terms (zero in this model, but handled generally) are added on host.
"""

import numpy as np
import ml_dtypes

import concourse.bass as bass
import concourse.tile as tile
from concourse import bacc, mybir
from concourse.bass_utils import run_bass_kernel_spmd

BF16 = ml_dtypes.bfloat16

H = 2048          # hidden dim
F = 8192          # ffn dim
E = 8             # routed experts
S = 2             # shared experts
B, SEQ = 2, 2048
T = B * SEQ       # 4096 tokens
P = 128
DO = H // P       # 16 d-blocks
FO = F // P       # 64 f-blocks
TSH = T // 8      # 512 shared tokens per core
NT = 512          # token tile (matmul moving dim)

_PROG_CACHE = {}
LAST_RESULT = None  # test.py introspection


def _t_chunks(C):
    # equal-width chunks <= NT: a narrow tail chunk pays the full
    # per-matmul issue floor for few columns and goes DMA-bound.
    n = -(-C // NT)
    base, rem = divmod(C, n)
    chunks = []
    t0 = 0
    for i in range(n):
        nt = base + (1 if i < rem else 0)
        chunks.append((t0, nt))
        t0 += nt
    return chunks


def _build_program(C):
    """One SPMD program, identical on all 8 cores."""
    nc = bacc.Bacc("TRN2", target_bir_lowering=False, debug=False)
    bf = mybir.dt.bfloat16
    f32 = mybir.dt.float32

    def inp(name, shape, dt=bf):
        return nc.dram_tensor(name, list(shape), dt, kind="ExternalInput").ap()

    def outp(name, shape, dt=f32):
        return nc.dram_tensor(name, list(shape), dt, kind="ExternalOutput").ap()

    xsT = inp("xsT", [P, DO, TSH])            # shared tokens, [di, do, t]
    xrT = inp("xrT", [P, DO, C])              # routed tokens for this expert
    w1s = inp("w1s", [S, FO, P, DO, P])       # (s, fo, di, do, fi)
    w2s = inp("w2s", [S, DO, P, FO, P])       # (s, do, fi, fo, di)
    w1e = inp("w1e", [FO, P, DO, P])
    w2e = inp("w2e", [DO, P, FO, P])
    b1s = inp("b1s", [P, S, FO], f32)         # (fi, s, fo)
    b1e = inp("b1e", [P, FO], f32)
    wgp = inp("wgp", [P, DO, E])              # (di, do, e)

    lg_o = outp("lg_o", [E, TSH])             # logits for this core's tokens
    ys_o = outp("ys_o", [S, DO, P, TSH])      # shared outputs, [d, t] layout
    yr_o = outp("yr_o", [DO, P, C])           # routed output

    with tile.TileContext(nc) as tc:
        with (
            tc.tile_pool(name="xpool", bufs=1) as xpool,
            tc.tile_pool(name="cpool", bufs=1) as cpool,
            tc.tile_pool(name="w1p", bufs=4) as w1p,
            tc.tile_pool(name="w2p", bufs=3) as w2p,
            tc.tile_pool(name="hp", bufs=FO) as hp,
            tc.tile_pool(name="yp", bufs=4) as yp,
            tc.tile_pool(name="psp", bufs=4, space="PSUM") as psp,
            tc.tile_pool(name="psg", bufs=1, space="PSUM") as psg,
        ):
            # resident activations / constants
            xs_sb = xpool.tile([P, DO, TSH], bf, tag="xs")
            # per-d-block loads: the first matmuls (logits/L1 at do=0) start
            # after ~128KB instead of waiting on the whole 2MB transfer
            for do in range(DO):
                nc.sync.dma_start(xs_sb[:, do, :], xsT[:, do, :])
            xr_sb = xpool.tile([P, DO, C], bf, tag="xr")
            b1s_sb = cpool.tile([P, S, FO], f32, tag="b1s")
            nc.sync.dma_start(b1s_sb[:], b1s[:])
            b1e_sb = cpool.tile([P, FO], f32, tag="b1e")
            nc.sync.dma_start(b1e_sb[:], b1e[:])
            wg_sb = cpool.tile([P, DO, E], bf, tag="wg")
            nc.sync.dma_start(wg_sb[:], wgp[:])

            # ---- router logits for this core's 512 tokens ----
            ps_g = psg.tile([E, TSH], f32)
            for do in range(DO):
                nc.tensor.matmul(
                    ps_g[:],
                    wg_sb[:, do, :],
                    xs_sb[:, do, :],
                    start=(do == 0),
                    stop=(do == DO - 1),
                )
            lg_sb = yp.tile([E, TSH], f32, tag="lg")
            nc.vector.tensor_copy(lg_sb[:], ps_g[:])
            nc.sync.dma_start(lg_o[:], lg_sb[:])

            # ---- expert-MLP units ----
            # unit = (w1 dram [FO,P,DO,P], w2 dram [DO,P,FO,P], bias AP fn,
            #         x tile, t0, nt, out dram slice fn)
            shared_units = [
                (
                    w1s[s],
                    w2s[s],
                    lambda fo, s=s: b1s_sb[:, s, fo : fo + 1],
                    xs_sb,
                    0,
                    TSH,
                    lambda do, s=s: ys_o[s, do],
                )
                for s in range(S)
            ]
            routed_units = [
                (
                    w1e,
                    w2e,
                    lambda fo: b1e_sb[:, fo : fo + 1],
                    xr_sb,
                    t0,
                    nt,
                    lambda do, t0=t0, nt=nt: yr_o[do, :, t0 : t0 + nt],
                )
                for t0, nt in _t_chunks(C)
            ]

            def emit_unit(w1d, w2d, bias_fn, x_sb, t0, nt, out_fn):
                h_tiles = []
                for fo in range(FO):
                    w1_t = w1p.tile([P, DO, P], bf, tag="w1")
                    nc.sync.dma_start(w1_t[:], w1d[fo])
                    ps = psp.tile([P, NT], f32, tag="ps")
                    for do in range(DO):
                        nc.tensor.matmul(
                            ps[:, :nt],
                            w1_t[:, do, :],
                            x_sb[:, do, t0 : t0 + nt],
                            start=(do == 0),
                            stop=(do == DO - 1),
                        )
                    h_t = hp.tile([P, NT], bf, tag="h")
                    nc.scalar.activation(
                        h_t[:, :nt],
                        ps[:, :nt],
                        mybir.ActivationFunctionType.Relu,
                        bias=bias_fn(fo),
                    )
                    h_tiles.append(h_t)
                for do in range(DO):
                    w2_t = w2p.tile([P, FO, P], bf, tag="w2")
                    nc.sync.dma_start(w2_t[:], w2d[do])
                    ps2 = psp.tile([P, NT], f32, tag="ps")
                    for fo in range(FO):
                        nc.tensor.matmul(
                            ps2[:, :nt],
                            w2_t[:, fo, :],
                            h_tiles[fo][:, :nt],
                            start=(fo == 0),
                            stop=(fo == FO - 1),
                        )
                    y_t = yp.tile([P, NT], f32, tag="y")
                    nc.vector.tensor_copy(y_t[:, :nt], ps2[:, :nt])
                    nc.sync.dma_start(out_fn(do), y_t[:, :nt])

            for u in shared_units:
                emit_unit(*u)
            # xr loads deferred and split per chunk so the 4.7MB transfer
            # neither queues ahead of shared weight streams nor contends
            # with them all at once
            for u in routed_units:
                t0, nt = u[4], u[5]
                nc.sync.dma_start(
                    xr_sb[:, :, t0 : t0 + nt], xrT[:, :, t0 : t0 + nt]
                )
                emit_unit(*u)

    nc.compile()
    return nc


def _get_program(C):
    if C not in _PROG_CACHE:
        _PROG_CACHE[C] = _build_program(C)
    return _PROG_CACHE[C]


def _pack_w1(w):
    # [H, F] -> (fo, di, do, fi); SBUF tile per fo: [di, do*fi] contiguous
    return np.ascontiguousarray(
        w.reshape(DO, P, FO, P).transpose(2, 1, 0, 3)
    )


def _pack_w2(w):
    # [F, H] -> (do, fi, fo, di)
    return np.ascontiguousarray(
        w.reshape(FO, P, DO, P).transpose(2, 1, 0, 3)
    )


def _pack_xT(xt_bf):
    # [n, H] bf16 -> [di, do, n]
    n = xt_bf.shape[0]
    return np.ascontiguousarray(
        xt_bf.T.reshape(DO, P, n).transpose(1, 0, 2)
    )


def kernel(
    x,
    ws_w1,
    ws_b1,
    ws_w2,
    ws_b2,
    we_w1,
    we_b1,
    we_w2,
    we_b2,
    wg,
    top_k,
    trace=False,
    run_kwargs=None,
    **_unused,
):
    global LAST_RESULT
    x = np.asarray(x, dtype=np.float32)
    ws_w1 = np.asarray(ws_w1, dtype=np.float32)
    ws_b1 = np.asarray(ws_b1, dtype=np.float32)
    ws_w2 = np.asarray(ws_w2, dtype=np.float32)
    ws_b2 = np.asarray(ws_b2, dtype=np.float32)
    we_w1 = np.asarray(we_w1, dtype=np.float32)
    we_b1 = np.asarray(we_b1, dtype=np.float32)
    we_w2 = np.asarray(we_w2, dtype=np.float32)
    we_b2 = np.asarray(we_b2, dtype=np.float32)
    wg = np.asarray(wg, dtype=np.float32)
    k = int(top_k)

    Bx, Sx, Hx = x.shape
    Tx = Bx * Sx
    assert (Tx, Hx) == (T, H), f"unexpected shape {x.shape}"
    xt = x.reshape(Tx, Hx)

    # ---- host routing decision (dispatch plan only; logits returned from HW)
    logits_h = xt @ wg
    m = logits_h.max(axis=1, keepdims=True)
    p = np.exp(logits_h - m)
    probs = p / p.sum(axis=1, keepdims=True)
    order = np.argsort(-probs, axis=1, kind="stable")[:, :k]
    gates = np.zeros_like(probs)
    np.put_along_axis(gates, order, np.take_along_axis(probs, order, 1), 1)
    sel = np.zeros((Tx, E), dtype=bool)
    np.put_along_axis(sel, order, True, 1)
    idx_e = [np.nonzero(sel[:, e])[0] for e in range(E)]
    n_e = [len(ix) for ix in idx_e]
    # capacity: equal 4-aligned chunks of <= NT covering the largest expert
    mx = max(max(n_e), 16)
    nch = -(-mx // NT)
    C = -(-mx // (4 * nch)) * 4 * nch

    # ---- pack per-core inputs
    xt_bf = xt.astype(BF16)
    w1s_h = np.stack([_pack_w1(ws_w1[s].astype(BF16)) for s in range(S)])
    w2s_h = np.stack([_pack_w2(ws_w2[s].astype(BF16)) for s in range(S)])
    b1s_h = np.ascontiguousarray(
        ws_b1.reshape(S, FO, P).transpose(2, 0, 1)
    ).astype(np.float32)
    wg_h = np.ascontiguousarray(
        wg.astype(BF16).reshape(DO, P, E).transpose(1, 0, 2)
    )

    in_maps = []
    for c in range(E):
        xr = np.zeros((C, H), dtype=BF16)
        xr[: n_e[c]] = xt_bf[idx_e[c]]
        in_maps.append(
            {
                "xsT": _pack_xT(xt_bf[c * TSH : (c + 1) * TSH]),
                "xrT": _pack_xT(xr),
                "w1s": w1s_h,
                "w2s": w2s_h,
                "w1e": _pack_w1(we_w1[c].astype(BF16)),
                "w2e": _pack_w2(we_w2[c].astype(BF16)),
                "b1s": b1s_h,
                "b1e": np.ascontiguousarray(
                    we_b1[c].reshape(FO, P).T
                ).astype(np.float32),
                "wgp": wg_h,
            }
        )

    nc = _get_program(C)
    res = run_bass_kernel_spmd(
        nc, in_maps, list(range(8)), trace=trace, **(run_kwargs or {})
    )
    LAST_RESULT = res

    # ---- combine on host
    out = np.zeros((Tx, H), dtype=np.float32)
    logits = np.empty((Tx, E), dtype=np.float32)
    for c in range(E):
        r = res.results[c]
        logits[c * TSH : (c + 1) * TSH] = r["lg_o"].T
        ys = r["ys_o"].sum(axis=0)  # [DO, P, TSH]
        out[c * TSH : (c + 1) * TSH] += (
            ys.reshape(H, TSH).T
        )
        if n_e[c]:
            yr = r["yr_o"].reshape(H, C).T[: n_e[c]]  # [n_e, H]
            out[idx_e[c]] += gates[idx_e[c], c : c + 1] * yr
    out += ws_b2.sum(axis=0)[None, :]
    out += gates @ we_b2
    return out.reshape(Bx, Sx, Hx), logits
